# revision 3
# baseline (speedup 1.0000x reference)
"""BitLinear (ternary weight quantization + linear) on 8 Trainium2 NeuronCores.

Math: out = (x @ w_q.T + b) * LAYER_SCALE, where
  beta = max(mean(|W|), eps)           (global scalar over the full W)
  w_q  = clip(round(W / beta), -1, 1) * beta   (ternary: beta * {-1, 0, +1})

Device strategy (per the column-parallel sharding hint, plus data-parallel):
  8 cores = 2 batch-shards (tokens) x 4 feature-shards (out_features).
  Each core: quantize its W shard to ternary {-1,0,+1} in bf16 (exact),
  then a bf16 matmul (x cast to bf16) accumulating fp32 in PSUM, then a
  fused scale+bias drain on the Scalar engine. Ternary weights are exact
  in bf16, so the only precision loss is the bf16 rounding of x (~1e-3).

Numerical care: round(W/beta) decisions near |W/beta| = 0.5 flip with the
last ulp of beta. beta is therefore computed on host with jax-CPU exactly
as the reference does, and the round-half-to-even decision is lowered to
an exact fp32 threshold compare |W| > c where c is the largest float32
with fl32(c/beta) <= 0.5 (verified bit-identical to the reference
quantization). On device the quantization is then two compares + a
subtract per element — no rounding-mode hazards.
"""

import math
from functools import lru_cache

import numpy as np

import concourse.bass as bass
import concourse.mybir as mybir
import concourse.tile as tile
from concourse import bacc
from concourse.bass import ts
from concourse.bass_utils import run_bass_kernel_spmd

P = 128
IN_FEATURES = 2048
OUT_FEATURES = 8192
N_TOKENS = 8192  # 4 * 2048
EPS = 1e-8
LAYER_SCALE = np.float32(1.0 / math.sqrt(IN_FEATURES))

S_WAYS = 2  # data-parallel over tokens
Q_WAYS = 4  # tensor-parallel over out_features
N_CORES = S_WAYS * Q_WAYS

F32 = mybir.dt.float32
BF16 = mybir.dt.bfloat16


@lru_cache(maxsize=4)
def build_nc(KI: int, OC: int, TC: int, TB: int = 512):
    """Per-core bass program.

    Inputs (per core):
      xt   [KI, TC] f32 : x^T shard (in_features x tokens)
      wt   [KI, OC] f32 : W^T shard (in_features x out_features)
      bvec [OC]     f32 : bias shard
      scl  [P, 1]   f32 : beta * LAYER_SCALE (broadcast)
      cut  [P, 1]   f32 : quantization threshold c (broadcast)
      ncut [P, 1]   f32 : -c (broadcast)
    Output:
      out  [OC, TC] f32 : (x @ w_q.T)^T shard, scaled and biased
    """
    assert KI % P == 0 and OC % P == 0 and TC % TB == 0
    K_TILES = KI // P
    M_TILES = OC // P
    T_BLOCKS = TC // TB

    nc = bacc.Bacc(None, target_bir_lowering=False, name="bitlinear")

    xt = nc.dram_tensor("xt", [KI, TC], F32, kind="ExternalInput")
    wt = nc.dram_tensor("wt", [KI, OC], F32, kind="ExternalInput")
    bvec = nc.dram_tensor("bvec", [OC], F32, kind="ExternalInput")
    scl = nc.dram_tensor("scl", [P, 1], F32, kind="ExternalInput")
    cut = nc.dram_tensor("cut", [P, 1], F32, kind="ExternalInput")
    ncut = nc.dram_tensor("ncut", [P, 1], F32, kind="ExternalInput")
    out = nc.dram_tensor("out", [OC, TC], F32, kind="ExternalOutput")

    with tile.TileContext(nc) as tc:
        with (
            tc.tile_pool(name="const", bufs=1) as cpool,
            tc.tile_pool(name="wq", bufs=1) as wqpool,
            tc.tile_pool(name="xb", bufs=2) as xbpool,
            tc.tile_pool(name="xs", bufs=4) as xspool,
            tc.tile_pool(name="ot", bufs=4) as opool,
            tc.tile_pool(name="ps", bufs=4, space="PSUM") as pspool,
        ):
            # --- constants ---
            cut_t = cpool.tile([P, 1], F32)
            ncut_t = cpool.tile([P, 1], F32)
            scl_t = cpool.tile([P, 1], F32)
            bt = cpool.tile([P, M_TILES], F32)
            bs = cpool.tile([P, M_TILES], F32)
            nc.sync.dma_start(cut_t[:], cut[:])
            nc.sync.dma_start(ncut_t[:], ncut[:])
            nc.sync.dma_start(scl_t[:], scl[:])
            # bias strided so bs[:, m] holds b[m*128 : (m+1)*128]
            nc.sync.dma_start(bt[:], bvec[:].rearrange("(m p) -> p m", p=P))
            nc.vector.tensor_scalar_mul(bs[:], bt[:], float(LAYER_SCALE))

            # --- quantize W shard to ternary bf16, K-major, resident ---
            wq = []  # wq[k] : [P, OC] bf16 with {-1, 0, +1}
            with (
                tc.tile_pool(name="wstage", bufs=3) as wspool,
                tc.tile_pool(name="qtmp", bufs=3) as qpool,
            ):
                for k in range(K_TILES):
                    wst = wspool.tile([P, OC], F32, tag="wst")
                    nc.sync.dma_start(wst[:], wt[ts(k, P), :])
                    pos = qpool.tile([P, OC], BF16, tag="pos")
                    neg = qpool.tile([P, OC], BF16, tag="neg")
                    wq_k = wqpool.tile([P, OC], BF16, tag=f"wq{k}")
                    nc.vector.tensor_scalar(
                        pos[:], wst[:], cut_t[:, 0:1], None, mybir.AluOpType.is_gt
                    )
                    nc.vector.tensor_scalar(
                        neg[:], wst[:], ncut_t[:, 0:1], None, mybir.AluOpType.is_lt
                    )
                    nc.vector.tensor_sub(wq_k[:], pos[:], neg[:])
                    wq.append(wq_k)

            # --- main loop: stream x blocks, matmul, fused drain ---
            for tb in range(T_BLOCKS):
                xb = []  # xb[k] : [P, TB] bf16
                for k in range(K_TILES):
                    xs = xspool.tile([P, TB], F32, tag="xs")
                    nc.sync.dma_start(xs[:], xt[ts(k, P), ts(tb, TB)])
                    xb_k = xbpool.tile([P, TB], BF16, tag=f"xb{k}")
                    nc.vector.tensor_copy(xb_k[:], xs[:])
                    xb.append(xb_k)

                for m in range(M_TILES):
                    ps = pspool.tile([P, TB], F32, tag="ps")
                    for k in range(K_TILES):
                        nc.tensor.matmul(
                            ps[:],
                            wq[k][:, ts(m, P)],
                            xb[k][:],
                            start=(k == 0),
                            stop=(k == K_TILES - 1),
                        )
                    ot = opool.tile([P, TB], F32, tag="ot")
                    # ot = psum * (beta * LAYER_SCALE) + b * LAYER_SCALE
                    nc.scalar.activation(
                        ot[:],
                        ps[:],
                        mybir.ActivationFunctionType.Identity,
                        bias=bs[:, m : m + 1],
                        scale=scl_t[:, 0:1],
                    )
                    nc.sync.dma_start(out[ts(m, P), ts(tb, TB)], ot[:])

    nc.compile()
    return nc


def _host_beta_cut(W: np.ndarray):
    """beta exactly as the (jax) reference computes it, plus the exact fp32
    threshold c reproducing round-half-to-even of W/beta near 0.5."""
    try:
        import jax
        import jax.numpy as jnp

        cpu = jax.local_devices(backend="cpu")[0]
        with jax.default_device(cpu):
            beta = np.float32(jnp.maximum(jnp.mean(jnp.abs(jnp.asarray(W))), EPS))
    except Exception:
        beta = np.float32(max(np.abs(W).astype(np.float64).mean(), EPS))

    v = np.float32(0.5) * beta  # exact (power-of-two scale)
    assert np.float32(v / beta) <= np.float32(0.5)
    while True:
        nv = np.nextafter(v, np.float32(np.inf))
        if np.float32(nv / beta) <= np.float32(0.5):
            v = nv
        else:
            break
    return beta, v


def kernel(x: np.ndarray, W: np.ndarray, b: np.ndarray) -> np.ndarray:
    out, _ = _run(x, W, b)
    return out


def _run(x, W, b, **spmd_kwargs):
    x = np.ascontiguousarray(np.asarray(x, dtype=np.float32))
    W = np.ascontiguousarray(np.asarray(W, dtype=np.float32))
    b = np.ascontiguousarray(np.asarray(b, dtype=np.float32))

    B, T, KI = x.shape
    OC_full, KI2 = W.shape
    assert KI == KI2 == IN_FEATURES and OC_full == OUT_FEATURES
    NT = B * T
    assert NT == N_TOKENS

    TC = NT // S_WAYS  # tokens per core
    OC = OUT_FEATURES // Q_WAYS  # out features per core

    beta, c = _host_beta_cut(W)
    S = np.float32(beta * LAYER_SCALE)
    scl = np.full((P, 1), S, dtype=np.float32)
    cut_a = np.full((P, 1), c, dtype=np.float32)
    ncut_a = np.full((P, 1), np.float32(-c), dtype=np.float32)

    xf = x.reshape(NT, KI)
    xt_s = [
        np.ascontiguousarray(xf[s * TC : (s + 1) * TC, :].T) for s in range(S_WAYS)
    ]
    wt_q = [
        np.ascontiguousarray(W[q * OC : (q + 1) * OC, :].T) for q in range(Q_WAYS)
    ]
    b_q = [
        np.ascontiguousarray(b[q * OC : (q + 1) * OC]) for q in range(Q_WAYS)
    ]

    in_maps = []
    for s in range(S_WAYS):
        for q in range(Q_WAYS):
            in_maps.append(
                {
                    "xt": xt_s[s],
                    "wt": wt_q[q],
                    "bvec": b_q[q],
                    "scl": scl,
                    "cut": cut_a,
                    "ncut": ncut_a,
                }
            )

    nc = build_nc(KI, OC, TC)
    res = run_bass_kernel_spmd(nc, in_maps, core_ids=list(range(N_CORES)), **spmd_kwargs)

    out_full = np.empty((NT, OUT_FEATURES), dtype=np.float32)
    for s in range(S_WAYS):
        for q in range(Q_WAYS):
            piece = res.results[s * Q_WAYS + q]["out"]  # [OC, TC]
            out_full[s * TC : (s + 1) * TC, q * OC : (q + 1) * OC] = piece.T
    return out_full.reshape(B, T, OUT_FEATURES), res


# revision 5
# speedup vs baseline: 1.0440x; 1.0440x over previous
"""BitLinear (ternary weight quantization + linear) on 8 Trainium2 NeuronCores.

Math: out = (x @ w_q.T + b) * LAYER_SCALE, where
  beta = max(mean(|W|), eps)           (global scalar over the full W)
  w_q  = clip(round(W / beta), -1, 1) * beta   (ternary: beta * {-1, 0, +1})

Device strategy (per the column-parallel sharding hint, plus data-parallel):
  8 cores = 2 batch-shards (tokens) x 4 feature-shards (out_features).
  Each core: quantize its W shard to ternary {-1,0,+1} in bf16 (exact),
  then a bf16 matmul (x cast to bf16) accumulating fp32 in PSUM, then a
  fused scale+bias drain on the Scalar engine. Ternary weights are exact
  in bf16, so the only precision loss is the bf16 rounding of x (~1e-3).

Numerical care: round(W/beta) decisions near |W/beta| = 0.5 flip with the
last ulp of beta. beta is therefore computed on host with jax-CPU exactly
as the reference does, and the round-half-to-even decision is lowered to
an exact fp32 threshold compare |W| > c where c is the largest float32
with fl32(c/beta) <= 0.5 (verified bit-identical to the reference
quantization). On device the quantization is then two compares + a
subtract per element — no rounding-mode hazards.
"""

import math
from functools import lru_cache

import numpy as np

import concourse.bass as bass
import concourse.mybir as mybir
import concourse.tile as tile
from concourse import bacc
from concourse.bass import ts
from concourse.bass_utils import run_bass_kernel_spmd

P = 128
IN_FEATURES = 2048
OUT_FEATURES = 8192
N_TOKENS = 8192  # 4 * 2048
EPS = 1e-8
LAYER_SCALE = np.float32(1.0 / math.sqrt(IN_FEATURES))

S_WAYS = 2  # data-parallel over tokens
Q_WAYS = 4  # tensor-parallel over out_features
N_CORES = S_WAYS * Q_WAYS

F32 = mybir.dt.float32
BF16 = mybir.dt.bfloat16


@lru_cache(maxsize=4)
def build_nc(KI: int, OC: int, TC: int, TB: int = 512):
    """Per-core bass program.

    Inputs (per core):
      xt   [KI, TC] f32 : x^T shard (in_features x tokens)
      wt   [KI, OC] f32 : W^T shard (in_features x out_features)
      bvec [OC]     f32 : bias shard
      scl  [P, 1]   f32 : beta * LAYER_SCALE (broadcast)
      cut  [P, 1]   f32 : quantization threshold c (broadcast)
      ncut [P, 1]   f32 : -c (broadcast)
    Output:
      out  [OC, TC] f32 : (x @ w_q.T)^T shard, scaled and biased
    """
    assert KI % P == 0 and OC % P == 0 and TC % TB == 0
    K_TILES = KI // P
    M_TILES = OC // P
    T_BLOCKS = TC // TB

    nc = bacc.Bacc(None, target_bir_lowering=False, name="bitlinear")

    xt = nc.dram_tensor("xt", [KI, TC], F32, kind="ExternalInput")
    wt = nc.dram_tensor("wt", [KI, OC], F32, kind="ExternalInput")
    bvec = nc.dram_tensor("bvec", [OC], F32, kind="ExternalInput")
    scl = nc.dram_tensor("scl", [P, 1], F32, kind="ExternalInput")
    cut = nc.dram_tensor("cut", [P, 1], F32, kind="ExternalInput")
    ncut = nc.dram_tensor("ncut", [P, 1], F32, kind="ExternalInput")
    out = nc.dram_tensor("out", [OC, TC], F32, kind="ExternalOutput")

    with tile.TileContext(nc) as tc:
        with (
            tc.tile_pool(name="const", bufs=1) as cpool,
            tc.tile_pool(name="wq", bufs=1) as wqpool,
            tc.tile_pool(name="xb", bufs=2) as xbpool,
            tc.tile_pool(name="xs", bufs=4) as xspool,
            tc.tile_pool(name="ot", bufs=4) as opool,
            tc.tile_pool(name="ps", bufs=4, space="PSUM") as pspool,
        ):
            # --- constants ---
            cut_t = cpool.tile([P, 1], F32)
            ncut_t = cpool.tile([P, 1], F32)
            scl_t = cpool.tile([P, 1], F32)
            bt = cpool.tile([P, M_TILES], F32)
            bs = cpool.tile([P, M_TILES], F32)
            nc.sync.dma_start(cut_t[:], cut[:])
            nc.sync.dma_start(ncut_t[:], ncut[:])
            nc.sync.dma_start(scl_t[:], scl[:])
            # bias strided so bs[:, m] holds b[m*128 : (m+1)*128]
            nc.sync.dma_start(bt[:], bvec[:].rearrange("(m p) -> p m", p=P))
            nc.vector.tensor_scalar_mul(bs[:], bt[:], float(LAYER_SCALE))

            # tb0's x loads + casts first: PE can start as soon as the first
            # weight chunks are quantized. Casts live on the Scalar engine
            # (with the psum drains); the Vector engine does quantization
            # only, so neither blocks the other.
            def load_x_block(tb):
                xb = []  # xb[k] : [P, TB] bf16
                for k in range(K_TILES):
                    xs = xspool.tile([P, TB], F32, tag="xs")
                    nc.sync.dma_start(xs[:], xt[ts(k, P), ts(tb, TB)])
                    xb_k = xbpool.tile([P, TB], BF16, tag=f"xb{k}")
                    nc.scalar.copy(xb_k[:], xs[:])
                    xb.append(xb_k)
                return xb

            xb0 = load_x_block(0)

            # --- quantize W shard to ternary bf16, chunk-major so the PE's
            # m-ascending consumption follows production order ---
            CHUNK = min(512, OC)
            N_CHUNKS = OC // CHUNK
            M_PER_CHUNK = CHUNK // P
            # wq[k][c] : [P, CHUNK] bf16 with {-1, 0, +1}
            wq = [[None] * N_CHUNKS for _ in range(K_TILES)]
            with (
                tc.tile_pool(name="wstage", bufs=6) as wspool,
                tc.tile_pool(name="qtmp", bufs=6) as qpool,
            ):
                for c in range(N_CHUNKS):
                    for k in range(K_TILES):
                        wst = wspool.tile([P, CHUNK], F32, tag="wst")
                        nc.sync.dma_start(wst[:], wt[ts(k, P), ts(c, CHUNK)])
                        neg = qpool.tile([P, CHUNK], F32, tag="neg")
                        wq_kc = wqpool.tile([P, CHUNK], BF16, tag=f"wq{k}_{c}")
                        nc.vector.tensor_scalar(
                            neg[:], wst[:], ncut_t[:, 0:1], None, mybir.AluOpType.is_lt
                        )
                        # wq = (W > c) - (W < -c)
                        nc.vector.scalar_tensor_tensor(
                            wq_kc[:],
                            wst[:],
                            cut_t[:, 0:1],
                            neg[:],
                            mybir.AluOpType.is_gt,
                            mybir.AluOpType.subtract,
                        )
                        wq[k][c] = wq_kc

            # --- main loop: stream x blocks, matmul, fused drain ---
            for tb in range(T_BLOCKS):
                xb = xb0 if tb == 0 else load_x_block(tb)

                for m in range(M_TILES):
                    c, mi = divmod(m, M_PER_CHUNK)
                    ps = pspool.tile([P, TB], F32, tag="ps")
                    for k in range(K_TILES):
                        nc.tensor.matmul(
                            ps[:],
                            wq[k][c][:, ts(mi, P)],
                            xb[k][:],
                            start=(k == 0),
                            stop=(k == K_TILES - 1),
                        )
                    ot = opool.tile([P, TB], F32, tag="ot")
                    # ot = psum * (beta * LAYER_SCALE) + b * LAYER_SCALE
                    nc.scalar.activation(
                        ot[:],
                        ps[:],
                        mybir.ActivationFunctionType.Identity,
                        bias=bs[:, m : m + 1],
                        scale=scl_t[:, 0:1],
                    )
                    nc.sync.dma_start(out[ts(m, P), ts(tb, TB)], ot[:])

    nc.compile()
    return nc


def _host_beta_cut(W: np.ndarray):
    """beta exactly as the (jax) reference computes it, plus the exact fp32
    threshold c reproducing round-half-to-even of W/beta near 0.5."""
    try:
        import jax
        import jax.numpy as jnp

        cpu = jax.local_devices(backend="cpu")[0]
        with jax.default_device(cpu):
            beta = np.float32(jnp.maximum(jnp.mean(jnp.abs(jnp.asarray(W))), EPS))
    except Exception:
        beta = np.float32(max(np.abs(W).astype(np.float64).mean(), EPS))

    v = np.float32(0.5) * beta  # exact (power-of-two scale)
    assert np.float32(v / beta) <= np.float32(0.5)
    while True:
        nv = np.nextafter(v, np.float32(np.inf))
        if np.float32(nv / beta) <= np.float32(0.5):
            v = nv
        else:
            break
    return beta, v


def kernel(x: np.ndarray, W: np.ndarray, b: np.ndarray) -> np.ndarray:
    out, _ = _run(x, W, b)
    return out


def _run(x, W, b, **spmd_kwargs):
    x = np.ascontiguousarray(np.asarray(x, dtype=np.float32))
    W = np.ascontiguousarray(np.asarray(W, dtype=np.float32))
    b = np.ascontiguousarray(np.asarray(b, dtype=np.float32))

    B, T, KI = x.shape
    OC_full, KI2 = W.shape
    assert KI == KI2 == IN_FEATURES and OC_full == OUT_FEATURES
    NT = B * T
    assert NT == N_TOKENS

    TC = NT // S_WAYS  # tokens per core
    OC = OUT_FEATURES // Q_WAYS  # out features per core

    beta, c = _host_beta_cut(W)
    S = np.float32(beta * LAYER_SCALE)
    scl = np.full((P, 1), S, dtype=np.float32)
    cut_a = np.full((P, 1), c, dtype=np.float32)
    ncut_a = np.full((P, 1), np.float32(-c), dtype=np.float32)

    xf = x.reshape(NT, KI)
    xt_s = [
        np.ascontiguousarray(xf[s * TC : (s + 1) * TC, :].T) for s in range(S_WAYS)
    ]
    wt_q = [
        np.ascontiguousarray(W[q * OC : (q + 1) * OC, :].T) for q in range(Q_WAYS)
    ]
    b_q = [
        np.ascontiguousarray(b[q * OC : (q + 1) * OC]) for q in range(Q_WAYS)
    ]

    in_maps = []
    for s in range(S_WAYS):
        for q in range(Q_WAYS):
            in_maps.append(
                {
                    "xt": xt_s[s],
                    "wt": wt_q[q],
                    "bvec": b_q[q],
                    "scl": scl,
                    "cut": cut_a,
                    "ncut": ncut_a,
                }
            )

    nc = build_nc(KI, OC, TC)
    res = run_bass_kernel_spmd(nc, in_maps, core_ids=list(range(N_CORES)), **spmd_kwargs)

    out_full = np.empty((NT, OUT_FEATURES), dtype=np.float32)
    for s in range(S_WAYS):
        for q in range(Q_WAYS):
            piece = res.results[s * Q_WAYS + q]["out"]  # [OC, TC]
            out_full[s * TC : (s + 1) * TC, q * OC : (q + 1) * OC] = piece.T
    return out_full.reshape(B, T, OUT_FEATURES), res
